# revision 11
# baseline (speedup 1.0000x reference)
"""Trainium2 Bass kernel for nn_Attention_39436389712179 (sparse_attention).

Sharding: 8-way tensor parallel over heads (2 heads / core).
 - wq/wk/wv/wky/wvy column-sharded by head; wo row-sharded; gate with heads.
 - q/k LayerNorm couples all 2048 channels -> per-core partial (sum, sumsq)
   stats + one tiny AllReduce ([6, R] f32).
 - Per-core partial outputs (rows x full D) are summed on the host.

Layout: feature-major ("T") activations [channels, rows]; all matmuls run
in float32r (f32 data, bf16-rate on PE, ~1.6e-4 rel err per matmul).
RoPE channels are deinterleaved (evens then odds per head) by permuting the
q/k/ky weight columns host-side, so the rotation becomes partition-block ops.
Softmax runs max-free (scores are O(1) after LN) with the row-sum computed by
an ones-vector matmul, and the 1/sum normalization applied to the PV output.
"""
import math
import sys
from contextlib import ExitStack

import numpy as np

sys.path.insert(0, "/opt/trn_rl_repo")

from concourse import bacc
import concourse.tile as tile
import concourse.mybir as mybir
from concourse.masks import make_identity

F32 = mybir.dt.float32
F32R = mybir.dt.float32r
AF = mybir.ActivationFunctionType
ALU = mybir.AluOpType

# Full problem config
B_F, S_F, D_F, H_F, HD_F, LY_F, DY_F = 2, 2048, 2048, 16, 128, 512, 2048
NCORES = 8
HPC = H_F // NCORES          # heads per core = 2
C = HPC * HD_F               # channels per core = 256
HHD = H_F * HD_F             # LayerNorm width = 2048
EPS_QK = 1e-5
EPS_KY = 1e-6

TRACE = False                # test.py sets True to collect exec time
_BUILD_CACHE = {}


def _cfg_full():
    return dict(B=B_F, S=S_F, D=D_F, LY=LY_F, DY=DY_F)


def build(cfg, bench_mode=False):
    B, S, D, LY, DY = cfg["B"], cfg["S"], cfg["D"], cfg["LY"], cfg["DY"]
    R = B * S
    RY = B * LY
    NDB = D // 128            # d-blocks for x projections
    NYB = DY // 128
    NST = R // 512            # 512-col tiles over all rows
    NYST = RY // 512
    NJ = S // 512             # q chunks per batch
    NT = S // 128             # self-attn key tiles per batch
    NTY = LY // 128           # cross-attn key tiles per batch
    assert R % 512 == 0 and RY % 512 == 0 and S % 512 == 0
    assert LY % 128 == 0 and LY <= 512

    nc = bacc.Bacc("TRN2", target_bir_lowering=False,
                   num_devices=1 if bench_mode else NCORES)

    xT = nc.dram_tensor("xT", [D, R], F32, kind="ExternalInput")
    yT = nc.dram_tensor("yT", [DY, RY], F32, kind="ExternalInput")
    cos2 = nc.dram_tensor("cos2", [128, S], F32, kind="ExternalInput")
    sin2 = nc.dram_tensor("sin2", [128, S], F32, kind="ExternalInput")
    wq_d = nc.dram_tensor("wq_sl", [D, C], F32, kind="ExternalInput")
    wk_d = nc.dram_tensor("wk_sl", [D, C], F32, kind="ExternalInput")
    wv_d = nc.dram_tensor("wv_sl", [D, C], F32, kind="ExternalInput")
    wky_d = nc.dram_tensor("wky_sl", [DY, C], F32, kind="ExternalInput")
    wvy_d = nc.dram_tensor("wvy_sl", [DY, C], F32, kind="ExternalInput")
    wo_d = nc.dram_tensor("wo_sl", [C, D], F32, kind="ExternalInput")
    gam_d = nc.dram_tensor("gam", [65, C], F32, kind="ExternalInput")
    bet_d = nc.dram_tensor("bet", [65, C], F32, kind="ExternalInput")
    gate_d = nc.dram_tensor("gate_sl", [65, 1], F32, kind="ExternalInput")

    out_d = nc.dram_tensor("out_part", [R, D], F32, kind="ExternalOutput")

    stats_sh = nc.dram_tensor("stats_sh", [6, R], F32,
                              addr_space="Local" if bench_mode else "Shared")

    with tile.TileContext(nc) as tc, ExitStack() as _top:
        if True:
            cp = _top.enter_context(tc.tile_pool(name="consts", bufs=1))
            dp = _top.enter_context(tc.tile_pool(name="dram", bufs=1, space="DRAM"))
            # ---- constants ----
            cos2_t = cp.tile([128, S], F32, tag="cos2")
            nc.sync.dma_start(cos2_t[:], cos2[:, :])
            sin2_t = cp.tile([128, S], F32, tag="sin2")
            nc.sync.dma_start(sin2_t[:], sin2[:, :])
            gam_t = cp.tile([65, C], F32R, tag="gam")
            nc.sync.dma_start(gam_t[:], gam_d[:, :].bitcast(F32R))
            bet_t = cp.tile([65, C], F32R, tag="bet")
            nc.sync.dma_start(bet_t[:], bet_d[:, :].bitcast(F32R))
            neg1_32 = cp.tile([65, 512], F32, tag="neg1_32")
            nc.vector.memset(neg1_32[:], -1.0)
            neg1 = cp.tile([65, 512], F32R, tag="neg1")
            nc.vector.tensor_copy(neg1[:], neg1_32[:])
            ones_col32 = cp.tile([1, 128], F32, tag="onc32")
            nc.vector.memset(ones_col32[:], 1.0)
            ones_col = cp.tile([1, 128], F32R, tag="onc")
            nc.vector.tensor_copy(ones_col[:], ones_col32[:])
            ones_row32 = cp.tile([128, 1], F32, tag="onr32")
            nc.vector.memset(ones_row32[:], 1.0)
            ones_row = cp.tile([128, 1], F32R, tag="onr")
            nc.vector.tensor_copy(ones_row[:], ones_row32[:])
            eps_t = cp.tile([65, 1], F32, tag="eps")
            nc.vector.memset(eps_t[:], EPS_QK)
            nc.vector.memset(eps_t[64:65, :], EPS_KY)
            gate_t = cp.tile([65, 1], F32, tag="gate")
            nc.sync.dma_start(gate_t[:], gate_d[:, :])
            g_t = cp.tile([65, 1], F32, tag="gtanh")
            nc.scalar.activation(g_t[:], gate_t[:], AF.Tanh)
            # LN coefficient tiles (filled in phase 1S)
            rs_t = cp.tile([65, R], F32R, tag="rs")
            mrs_t = cp.tile([65, R], F32R, tag="mrs")

            # ---- DRAM scratch ----
            q_raw_dr = dp.tile([C, R], F32, tag="q_raw")
            k_raw_dr = dp.tile([C, R], F32, tag="k_raw")
            yk_raw_dr = dp.tile([C, RY], F32, tag="yk_raw")
            vT_dr = dp.tile([C, R], F32, tag="vT")
            yvT_dr = dp.tile([C, RY], F32, tag="yvT")
            v_dr = dp.tile([R, C], F32, tag="v")
            yv_dr = dp.tile([RY, C], F32, tag="yv")
            o_dr = dp.tile([C, R], F32, tag="o")
            stats_dr = dp.tile([6, R], F32, tag="stats")

            # =================== PHASE 1: projections + stats ===============
            with ExitStack() as _s1:
                wp = _s1.enter_context(tc.tile_pool(name="wx", bufs=1))
                xp = _s1.enter_context(tc.tile_pool(name="xt", bufs=3))
                rawp = _s1.enter_context(tc.tile_pool(name="raw", bufs=6))
                sqp = _s1.enter_context(tc.tile_pool(name="sq", bufs=3))
                smallp = _s1.enter_context(tc.tile_pool(name="small", bufs=6))
                pps = _s1.enter_context(tc.tile_pool(name="pps", bufs=6, space="PSUM"))
                stps = _s1.enter_context(tc.tile_pool(name="stps", bufs=2, space="PSUM"))
                wq_sb = wp.tile([128, NDB * C], F32R, tag="wq")
                nc.sync.dma_start(
                    wq_sb[:].rearrange("p (n c) -> p n c", n=NDB),
                    wq_d[:, :].rearrange("(n p) c -> p n c", p=128).bitcast(F32R))
                wk_sb = wp.tile([128, NDB * C], F32R, tag="wk")
                nc.sync.dma_start(
                    wk_sb[:].rearrange("p (n c) -> p n c", n=NDB),
                    wk_d[:, :].rearrange("(n p) c -> p n c", p=128).bitcast(F32R))
                wv_sb = wp.tile([128, NDB * C], F32R, tag="wv")
                nc.sync.dma_start(
                    wv_sb[:].rearrange("p (n c) -> p n c", n=NDB),
                    wv_d[:, :].rearrange("(n p) c -> p n c", p=128).bitcast(F32R))
                wky_sb = wp.tile([128, NYB * C], F32R, tag="wky")
                nc.sync.dma_start(
                    wky_sb[:].rearrange("p (n c) -> p n c", n=NYB),
                    wky_d[:, :].rearrange("(n p) c -> p n c", p=128).bitcast(F32R))
                wvy_sb = wp.tile([128, NYB * C], F32R, tag="wvy")
                nc.sync.dma_start(
                    wvy_sb[:].rearrange("p (n c) -> p n c", n=NYB),
                    wvy_d[:, :].rearrange("(n p) c -> p n c", p=128).bitcast(F32R))

                def proj_tile(src_dr, w_list, st, ndb, raw_specs):
                    """One 512-col tile of projections.

                    w_list: [(w_sb, psum_tiles[2])], raw_specs: list of
                    (psum_pair, spill_dr, stat_rows or None) per projection.
                    """
                    col = st * 512
                    for dblk in range(ndb):
                        xt = xp.tile([128, 512], F32R, tag="xt")
                        nc.sync.dma_start(
                            xt[:],
                            src_dr[dblk * 128:(dblk + 1) * 128,
                                   col:col + 512].bitcast(F32R))
                        for w_sb, pst in w_list:
                            for cb in range(2):
                                nc.tensor.matmul(
                                    pst[cb][:],
                                    w_sb[:, dblk * C + cb * 128:
                                         dblk * C + cb * 128 + 128],
                                    xt[:],
                                    start=(dblk == 0), stop=(dblk == ndb - 1))
                    for pst, spill_dr, stat_rows, use_act in raw_specs:
                        if stat_rows is None:
                            for cb in range(2):
                                vsb = rawp.tile([128, 512], F32, tag="raw")
                                nc.scalar.copy(vsb[:], pst[cb][:])
                                nc.sync.dma_start(
                                    spill_dr[cb * 128:(cb + 1) * 128,
                                             col:col + 512], vsb[:])
                        else:
                            st_sum = stps.tile([1, 512], F32, tag="stat")
                            st_sq = stps.tile([1, 512], F32, tag="stat")
                            for cb in range(2):
                                raw = rawp.tile([128, 512], F32R, tag="raw")
                                nc.vector.tensor_copy(raw[:], pst[cb][:])
                                nc.sync.dma_start(
                                    spill_dr[cb * 128:(cb + 1) * 128,
                                             col:col + 512],
                                    raw[:].bitcast(F32))
                                nc.tensor.matmul(st_sum[:], ones_row[:], raw[:],
                                                 start=(cb == 0), stop=(cb == 1))
                                sq = sqp.tile([128, 512], F32R, tag="sq")
                                nc.scalar.activation(sq[:], raw[:].bitcast(F32),
                                                     AF.Square)
                                nc.tensor.matmul(st_sq[:], ones_row[:], sq[:],
                                                 start=(cb == 0), stop=(cb == 1))
                            r0, r1 = stat_rows
                            s0 = smallp.tile([1, 512], F32, tag="small")
                            nc.vector.tensor_copy(s0[:], st_sum[:])
                            nc.sync.dma_start(stats_dr[r0:r0 + 1, col:col + 512],
                                              s0[:])
                            s1 = smallp.tile([1, 512], F32, tag="small")
                            nc.vector.tensor_copy(s1[:], st_sq[:])
                            nc.sync.dma_start(stats_dr[r1:r1 + 1, col:col + 512],
                                              s1[:])

                for st in range(NST):
                    qps = [pps.tile([128, 512], F32, tag="proj", name="projp") for _ in range(2)]
                    kps = [pps.tile([128, 512], F32, tag="proj", name="projp") for _ in range(2)]
                    vps = [pps.tile([128, 512], F32, tag="proj", name="projp") for _ in range(2)]
                    proj_tile(xT, [(wq_sb, qps), (wk_sb, kps), (wv_sb, vps)],
                              st, NDB,
                              [(qps, q_raw_dr, (0, 1), False),
                               (kps, k_raw_dr, (2, 3), False),
                               (vps, vT_dr, None, True)])
                for st in range(NYST):
                    ykps = [pps.tile([128, 512], F32, tag="proj", name="projp") for _ in range(2)]
                    yvps = [pps.tile([128, 512], F32, tag="proj", name="projp") for _ in range(2)]
                    proj_tile(yT, [(wky_sb, ykps), (wvy_sb, yvps)], st, NYB,
                              [(ykps, yk_raw_dr, (4, 5), False),
                               (yvps, yvT_dr, None, True)])
                # zero-fill unused y-stat columns
                if RY < R:
                    z = smallp.tile([1, 512], F32, tag="small")
                    nc.vector.memset(z[:], 0.0)
                    for col in range(RY, R, 512):
                        nc.sync.dma_start(stats_dr[4:5, col:col + 512], z[:])
                        nc.sync.dma_start(stats_dr[5:6, col:col + 512], z[:])

            # stats AllReduce (overlaps with the v transposes below)
            if bench_mode:
                nc.sync.dma_start(stats_sh[:, :], stats_dr[:])
            else:
                nc.gpsimd.collective_compute(
                    "AllReduce", ALU.add,
                    replica_groups=[list(range(NCORES))],
                    ins=[stats_dr[:].opt()], outs=[stats_sh[:, :].opt()])

            # =================== PHASE 1T: transpose v / yv =================
            with ExitStack() as _s2:
                trinp = _s2.enter_context(tc.tile_pool(name="trin", bufs=4))
                troutp = _s2.enter_context(tc.tile_pool(name="trout", bufs=4))
                trpsp = _s2.enter_context(tc.tile_pool(name="trps", bufs=2, space="PSUM"))
                ident = trinp.tile([128, 128], F32, tag="ident")
                make_identity(nc, ident[:])
                for src, dst, rows in ((vT_dr, v_dr, R), (yvT_dr, yv_dr, RY)):
                    for cb in range(2):
                        for rb in range(rows // 128):
                            tin = trinp.tile([128, 128], F32, tag="trin")
                            nc.sync.dma_start(
                                tin[:], src[cb * 128:(cb + 1) * 128,
                                            rb * 128:(rb + 1) * 128])
                            tps = trpsp.tile([128, 128], F32, tag="trps")
                            nc.tensor.transpose(tps[:], tin[:], ident[:])
                            tout = troutp.tile([128, 128], F32, tag="trout")
                            if rb % 2 == 0:
                                nc.vector.tensor_copy(tout[:], tps[:])
                            else:
                                nc.scalar.copy(tout[:], tps[:])
                            nc.sync.dma_start(
                                dst[rb * 128:(rb + 1) * 128,
                                    cb * 128:(cb + 1) * 128], tout[:])

            # =================== PHASE 1S: LN statistics ====================
            with tc.tile_pool(name="statm", bufs=1) as smp:
                sums_t = smp.tile([65, R], F32, tag="sums")
                nc.vector.memset(sums_t[:], 1.0)
                sq_t = smp.tile([65, R], F32, tag="sqs")
                nc.vector.memset(sq_t[:], 1.0)
                for i, row in enumerate((0, 2, 4)):
                    nc.sync.dma_start(sums_t[32 * i:32 * i + 1, :],
                                      stats_sh[row:row + 1, :])
                for i, row in enumerate((1, 3, 5)):
                    nc.sync.dma_start(sq_t[32 * i:32 * i + 1, :],
                                      stats_sh[row:row + 1, :])
                mu = smp.tile([65, R], F32, tag="mu")
                nc.scalar.mul(mu[:], sums_t[:], 1.0 / HHD)
                mu2 = smp.tile([65, R], F32, tag="mu2")
                nc.vector.tensor_mul(mu2[:], mu[:], mu[:])
                var = smp.tile([65, R], F32, tag="var")
                nc.vector.scalar_tensor_tensor(
                    var[:], sq_t[:], 1.0 / HHD, mu2[:],
                    op0=ALU.mult, op1=ALU.subtract)
                sig = smp.tile([65, R], F32, tag="sig")
                nc.scalar.activation(sig[:], var[:], AF.Sqrt,
                                     bias=eps_t[:, 0:1], scale=1.0)
                rs32 = smp.tile([65, R], F32, tag="rs32")
                nc.vector.reciprocal(rs32[:], sig[:])
                nc.vector.tensor_copy(rs_t[:], rs32[:])
                nc.vector.tensor_mul(mrs_t[:], mu[:], rs32[:])

            # =================== PHASE 2: attention =========================
            with ExitStack() as _s3:
                bigp = _s3.enter_context(tc.tile_pool(name="big", bufs=2))
                ykfp = _s3.enter_context(tc.tile_pool(name="ykf", bufs=2))
                lnp = _s3.enter_context(tc.tile_pool(name="lnraw", bufs=4))
                tmpp = _s3.enter_context(tc.tile_pool(name="lntmp", bufs=6))
                vp = _s3.enter_context(tc.tile_pool(name="vtl", bufs=2 * NT + 4))
                yvp = _s3.enter_context(tc.tile_pool(name="yvtl", bufs=2 * NTY + 2))
                ptp = _s3.enter_context(tc.tile_pool(name="ptile", bufs=3))
                obp = _s3.enter_context(tc.tile_pool(name="osb", bufs=4))
                rcp = _s3.enter_context(tc.tile_pool(name="rcs", bufs=6))
                coefp = _s3.enter_context(tc.tile_pool(name="coefps", bufs=2, space="PSUM"))
                sp_ = _s3.enter_context(tc.tile_pool(name="sps", bufs=2, space="PSUM"))
                OpsP = _s3.enter_context(tc.tile_pool(name="Ops", bufs=1, space="PSUM"))
                O2psP = _s3.enter_context(tc.tile_pool(name="O2ps", bufs=1, space="PSUM"))
                sumP = _s3.enter_context(tc.tile_pool(name="sums", bufs=1, space="PSUM"))
                sum2P = _s3.enter_context(tc.tile_pool(name="sums2", bufs=1, space="PSUM"))
                def ln_chunk(dst, dst_col, src_dr, base, hl, col0, j, do_rope):
                    col = col0 + j * 512
                    hs = hl * 128
                    raw = lnp.tile([128, 512], F32, tag="lnraw")
                    nc.sync.dma_start(raw[:],
                                      src_dr[hs:hs + 128, col:col + 512])
                    a_ps = coefp.tile([128, 512], F32, tag="coef")
                    nc.tensor.matmul(a_ps[:], gam_t[base:base + 1, hs:hs + 128],
                                     rs_t[base:base + 1, col:col + 512],
                                     start=True, stop=True)
                    b_ps = coefp.tile([128, 512], F32, tag="coef")
                    nc.tensor.matmul(b_ps[:], gam_t[base:base + 1, hs:hs + 128],
                                     mrs_t[base:base + 1, col:col + 512],
                                     start=True, stop=False)
                    nc.tensor.matmul(b_ps[:], bet_t[base:base + 1, hs:hs + 128],
                                     neg1[base:base + 1, 0:512],
                                     start=False, stop=True)
                    t1 = tmpp.tile([128, 512], F32, tag="lntmp")
                    nc.vector.tensor_mul(t1[:], raw[:], a_ps[:])
                    if not do_rope:
                        nc.vector.tensor_sub(dst[:, dst_col:dst_col + 512],
                                             t1[:], b_ps[:])
                        return
                    qln = tmpp.tile([128, 512], F32, tag="lntmp")
                    nc.vector.tensor_sub(qln[:], t1[:], b_ps[:])
                    # Deinterleaved RoPE: halves e=[0:64], o=[64:128].
                    # Each DVE op keeps both inputs at the same base
                    # partition (walrus constraint); outputs may shift.
                    cs = cos2_t[:, j * 512:(j + 1) * 512]
                    sn = sin2_t[:, j * 512:(j + 1) * 512]
                    m1e = tmpp.tile([64, 512], F32, tag="lnh")
                    nc.vector.tensor_mul(m1e[:], qln[0:64, :], cs[0:64, :])
                    m1o = tmpp.tile([64, 512], F32, tag="lnh")
                    nc.vector.tensor_mul(m1o[:], qln[64:128, :], cs[64:128, :])
                    m2e = tmpp.tile([64, 512], F32, tag="lnh")
                    nc.vector.tensor_mul(m2e[:], qln[0:64, :], sn[0:64, :])
                    m2o = tmpp.tile([64, 512], F32, tag="lnh")
                    nc.vector.tensor_mul(m2o[:], qln[64:128, :], sn[64:128, :])
                    nc.vector.tensor_sub(dst[0:64, dst_col:dst_col + 512],
                                         m1e[:], m2o[:])
                    nc.vector.tensor_add(dst[64:128, dst_col:dst_col + 512],
                                         m2e[:], m1o[:])

                for b in range(B):
                    for hl in range(HPC):
                        hs = hl * 128
                        q_f = bigp.tile([128, S], F32R, tag="qf")
                        k_f = bigp.tile([128, S], F32R, tag="kf")
                        yk_f = ykfp.tile([128, LY], F32R, tag="ykf")
                        for j in range(NJ):
                            ln_chunk(q_f, j * 512, q_raw_dr, 0, hl, b * S, j,
                                     True)
                            ln_chunk(k_f, j * 512, k_raw_dr, 32, hl, b * S, j,
                                     True)
                        # yk LN (LY <= 512: single chunk)
                        col = b * LY
                        raw = lnp.tile([128, LY], F32, tag="lnrawy")
                        nc.sync.dma_start(raw[:],
                                          yk_raw_dr[hs:hs + 128, col:col + LY])
                        a_ps = coefp.tile([128, LY], F32, tag="coef")
                        nc.tensor.matmul(a_ps[:], gam_t[64:65, hs:hs + 128],
                                         rs_t[64:65, col:col + LY],
                                         start=True, stop=True)
                        b_ps = coefp.tile([128, LY], F32, tag="coef")
                        nc.tensor.matmul(b_ps[:], gam_t[64:65, hs:hs + 128],
                                         mrs_t[64:65, col:col + LY],
                                         start=True, stop=False)
                        nc.tensor.matmul(b_ps[:], bet_t[64:65, hs:hs + 128],
                                         neg1[64:65, 0:LY],
                                         start=False, stop=True)
                        t1 = tmpp.tile([128, LY], F32, tag="lntmpy")
                        nc.vector.tensor_mul(t1[:], raw[:], a_ps[:])
                        nc.vector.tensor_sub(yk_f[:], t1[:], b_ps[:])

                        vt = []
                        for t in range(NT):
                            v_t = vp.tile([128, 128], F32R, tag="v")
                            nc.sync.dma_start(
                                v_t[:],
                                v_dr[b * S + t * 128:b * S + (t + 1) * 128,
                                     hs:hs + 128].bitcast(F32R))
                            vt.append(v_t)
                        yvt = []
                        for t in range(NTY):
                            yv_t = yvp.tile([128, 128], F32R, tag="yv")
                            nc.sync.dma_start(
                                yv_t[:],
                                yv_dr[b * LY + t * 128:b * LY + (t + 1) * 128,
                                      hs:hs + 128].bitcast(F32R))
                            yvt.append(yv_t)

                        for j in range(NJ):
                            qsl = q_f[:, j * 512:(j + 1) * 512]
                            O_ps = OpsP.tile([128, 512], F32, tag="O")
                            Os_ps = sumP.tile([1, 512], F32, tag="sum")
                            for t in range(NT):
                                s_ps = sp_.tile([128, 512], F32, tag="s")
                                nc.tensor.matmul(
                                    s_ps[:], k_f[:, t * 128:(t + 1) * 128],
                                    qsl, start=True, stop=True)
                                p_t = ptp.tile([128, 512], F32R, tag="p")
                                nc.scalar.activation(p_t[:], s_ps[:], AF.Exp)
                                nc.tensor.matmul(O_ps[:], vt[t][:], p_t[:],
                                                 start=(t == 0),
                                                 stop=(t == NT - 1))
                                nc.tensor.matmul(Os_ps[:], ones_row[:], p_t[:],
                                                 start=(t == 0),
                                                 stop=(t == NT - 1))
                            O2_ps = O2psP.tile([128, 512], F32, tag="O2")
                            O2s_ps = sum2P.tile([1, 512], F32, tag="sum2")
                            for t in range(NTY):
                                s_ps = sp_.tile([128, 512], F32, tag="s")
                                nc.tensor.matmul(
                                    s_ps[:], yk_f[:, t * 128:(t + 1) * 128],
                                    qsl, start=True, stop=True)
                                p_t = ptp.tile([128, 512], F32R, tag="p")
                                nc.scalar.activation(p_t[:], s_ps[:], AF.Exp)
                                nc.tensor.matmul(O2_ps[:], yvt[t][:], p_t[:],
                                                 start=(t == 0),
                                                 stop=(t == NTY - 1))
                                nc.tensor.matmul(O2s_ps[:], ones_row[:],
                                                 p_t[:], start=(t == 0),
                                                 stop=(t == NTY - 1))
                            rc1 = rcp.tile([1, 512], F32R, tag="rc")
                            with nc.allow_low_precision(
                                    reason="f32r holds full f32 bits"):
                                nc.vector.reciprocal(rc1[:], Os_ps[:])
                            rc2_32 = rcp.tile([1, 512], F32, tag="rc32")
                            nc.vector.reciprocal(rc2_32[:], O2s_ps[:])
                            rc2 = rcp.tile([1, 512], F32R, tag="rc")
                            nc.vector.tensor_scalar(
                                out=rc2[:], in0=rc2_32[:],
                                scalar1=g_t[32 * hl:32 * hl + 1, 0:1], scalar2=None,
                                op0=ALU.mult)
                            r1_ps = sp_.tile([128, 512], F32, tag="s")
                            nc.tensor.matmul(r1_ps[:], ones_col[:], rc1[:],
                                             start=True, stop=True)
                            r2_ps = sp_.tile([128, 512], F32, tag="s")
                            nc.tensor.matmul(r2_ps[:], ones_col[:], rc2[:],
                                             start=True, stop=True)
                            r1_sb = tmpp.tile([128, 512], F32, tag="lntmp")
                            nc.scalar.copy(r1_sb[:], r1_ps[:])
                            r2_sb = tmpp.tile([128, 512], F32, tag="lntmp")
                            nc.scalar.copy(r2_sb[:], r2_ps[:])
                            o1 = obp.tile([128, 512], F32, tag="ob")
                            nc.vector.tensor_mul(o1[:], O_ps[:], r1_sb[:])
                            o2 = obp.tile([128, 512], F32, tag="ob")
                            nc.vector.tensor_mul(o2[:], O2_ps[:], r2_sb[:])
                            of = obp.tile([128, 512], F32, tag="ob")
                            nc.vector.tensor_add(of[:], o1[:], o2[:])
                            nc.sync.dma_start(
                                o_dr[hs:hs + 128,
                                     b * S + j * 512:b * S + (j + 1) * 512],
                                of[:])

            # =================== PHASE 3: output projection =================
            with ExitStack() as _s4:
                wop = _s4.enter_context(tc.tile_pool(name="wo", bufs=1))
                otp = _s4.enter_context(tc.tile_pool(name="ot", bufs=6))
                outp = _s4.enter_context(tc.tile_pool(name="outs", bufs=4))
                ops3 = _s4.enter_context(tc.tile_pool(name="ops3", bufs=2, space="PSUM"))
                wo_sb = wop.tile([128, 2 * D], F32R, tag="wo")
                nc.sync.dma_start(
                    wo_sb[:].rearrange("p (n d) -> p n d", n=2),
                    wo_d[:, :].rearrange("(n p) d -> p n d", p=128).bitcast(F32R))
                for rt in range(R // 128):
                    o_ts = []
                    for cb in range(2):
                        o_t = otp.tile([128, 128], F32R, tag="ot")
                        nc.sync.dma_start(
                            o_t[:],
                            o_dr[cb * 128:(cb + 1) * 128,
                                 rt * 128:(rt + 1) * 128].bitcast(F32R))
                        o_ts.append(o_t)
                    for oc in range(D // 512):
                        ps = ops3.tile([128, 512], F32, tag="out")
                        for cb in range(2):
                            nc.tensor.matmul(
                                ps[:], o_ts[cb][:],
                                wo_sb[:, cb * D + oc * 512:
                                      cb * D + (oc + 1) * 512],
                                start=(cb == 0), stop=(cb == 1))
                        ob_ = outp.tile([128, 512], F32, tag="outsb")
                        if oc % 2 == 0:
                            nc.scalar.copy(ob_[:], ps[:])
                        else:
                            nc.vector.tensor_copy(ob_[:], ps[:])
                        nc.sync.dma_start(
                            out_d[rt * 128:(rt + 1) * 128,
                                  oc * 512:(oc + 1) * 512], ob_[:])

    nc.compile()
    return nc


def _perm_for_core(c):
    idx = []
    for h in (HPC * c + i for i in range(HPC)):
        base = h * HD_F
        idx.extend(base + np.arange(0, HD_F, 2))
        idx.extend(base + np.arange(1, HD_F, 2))
    return np.array(idx)


def make_in_maps(cfg, inputs):
    B, S, D, LY, DY = cfg["B"], cfg["S"], cfg["D"], cfg["LY"], cfg["DY"]
    R, RY = B * S, B * LY
    f32 = np.float32
    x = np.asarray(inputs["x"], f32)
    y = np.asarray(inputs["y"], f32)
    fc = np.asarray(inputs["freqs_cis"], f32)      # [S, 64, 2]
    wq = np.asarray(inputs["wq"], f32)
    wk = np.asarray(inputs["wk"], f32)
    wv = np.asarray(inputs["wv"], f32)
    wo = np.asarray(inputs["wo"], f32)
    wky = np.asarray(inputs["wky"], f32)
    wvy = np.asarray(inputs["wvy"], f32)
    gate = np.asarray(inputs["gate"], f32)
    qn_w = np.asarray(inputs["qn_w"], f32)
    qn_b = np.asarray(inputs["qn_b"], f32)
    kn_w = np.asarray(inputs["kn_w"], f32)
    kn_b = np.asarray(inputs["kn_b"], f32)
    kyn_w = np.asarray(inputs["kyn_w"], f32)
    kyn_b = np.asarray(inputs["kyn_b"], f32)

    xT = np.ascontiguousarray(x.reshape(R, D).T)
    yT = np.ascontiguousarray(y.reshape(RY, DY).T)
    cosv = fc[:, :, 0].T                           # [64, S]
    sinv = fc[:, :, 1].T
    cos2 = np.ascontiguousarray(np.concatenate([cosv, cosv], axis=0))
    sin2 = np.ascontiguousarray(np.concatenate([sinv, sinv], axis=0))
    scale = 1.0 / math.sqrt(HD_F)

    in_maps = []
    for c in range(NCORES):
        perm = _perm_for_core(c)
        nat = np.arange(c * C, (c + 1) * C)
        gam = np.zeros((65, C), f32)
        bet = np.zeros((65, C), f32)
        gam[0] = qn_w[perm] * scale
        bet[0] = qn_b[perm] * scale
        gam[32] = kn_w[perm]
        bet[32] = kn_b[perm]
        gam[64] = kyn_w[perm]
        bet[64] = kyn_b[perm]
        gate_65 = np.zeros((65, 1), f32)
        for i in range(HPC):
            gate_65[32 * i, 0] = gate[HPC * c + i]
        in_maps.append(dict(
            xT=xT, yT=yT, cos2=cos2, sin2=sin2,
            wq_sl=np.ascontiguousarray(wq[:, perm]),
            wk_sl=np.ascontiguousarray(wk[:, perm]),
            wv_sl=np.ascontiguousarray(wv[:, nat]),
            wky_sl=np.ascontiguousarray(wky[:, perm]),
            wvy_sl=np.ascontiguousarray(wvy[:, nat]),
            wo_sl=np.ascontiguousarray(wo[nat, :]),
            gam=gam, bet=bet,
            gate_sl=gate_65,
        ))
    return in_maps


def kernel(**inputs):
    from concourse.bass_utils import run_bass_kernel_spmd
    cfg = _cfg_full()
    key = tuple(sorted(cfg.items()))
    if key not in _BUILD_CACHE:
        _BUILD_CACHE[key] = build(cfg)
    nc = _BUILD_CACHE[key]
    in_maps = make_in_maps(cfg, inputs)
    try:
        res = run_bass_kernel_spmd(nc, in_maps, list(range(NCORES)),
                                   trace=TRACE)
    except ModuleNotFoundError:
        res = run_bass_kernel_spmd(nc, in_maps, list(range(NCORES)))
    acc = np.zeros((cfg["B"] * cfg["S"], cfg["D"]), np.float64)
    for r in res.results:
        acc += r["out_part"].astype(np.float64)
    out = acc.astype(np.float32).reshape(cfg["B"], cfg["S"], cfg["D"])
    kernel._last_result = res
    return out


kernel._last_result = None
